# revision 36
# baseline (speedup 1.0000x reference)
"""AnchorPatchPooling Trainium2 kernel.

Math (per sample n, channel c, part p):
  out[n,c,p] = sum_{k: lab[k]=p} feats[n,c,k]*vm[n,k] / max(count[n,p],1)
             + where(patch_count[p]>0, max(-100, max_{k: lab[k]=p} feats[n,c,k]), 0)

Design (PE-matmul mean + transposed-layout max):
 - Data-parallel over n across 8 cores (4 samples/core), no collectives.
 - Host re-lays feats per sample as fT[k, c]: k sorted part-major (parts
   ordered by chunk count), each part's segment padded with -100 rows to
   a multiple of 128, swizzled to [128, nch, 256] chunks, bf16.
 - MEAN on the TensorEngine: host builds M[k, p] = vm[n,k]/max(cnt,1)
   one-hot bf16 (pad rows 0).  Per 128-row chunk k: matmul(psum[16,256],
   lhsT=M_chunk[128,16], rhs=fT_chunk[128,256], accumulate) - the
   partition-dim contraction IS the segment scatter-add, and the mask
   multiply + count division ride along in M for free.  ACT evacuates
   psum, 2 small PE transposes give [c,p].
 - MAX on DVE+PE: per part, sequential wide bf16 tensor_tensor folds
   (2x_1P mode) collapse its chunks onto one [128,256] acc slot; PE
   transposes each [128k,128c] block to bf16 PSUM; 1x tensor_reduce
   per (sample, cb) over [p,t,64] finishes across the k-remnant, split
   in two (early pairs mid-stream, rest on the tail) since the DVE
   queue is FIFO.
 - One fused scalar_tensor_tensor does (max clamp -100) + meanT.
 - Pipelining (the DVE queue paces the kernel; DMA ~48us busy/engine,
   DVE ~56-62us busy):
   * slabs are runs of equal chunk-count (split >max_run) so each fold
     op is a flat contiguous 2D slice across all the slab's pairs;
     a 1-pair first slab + a 2-chunk first sub-DMA start folds ~9.5us
     in (Tile subtile deps release ops as their byte-range lands);
   * ft slabs ride the sync HWDGE ring; ident + all 4 M tiles (issued
     up-front) + out DMAs ride the scalar ring, so the per-sample mean
     evac COPY on ACT can't head-of-line-block later descriptor gens;
   * balanced slab sizes ([1,2,3,2] pairs) avoid a 2.9MB monster slab
     whose delivery Tile reorders behind the next sample's slabs.
 - Measured: ~77-81us (exec time varies with a bimodal device clock
   state; engine-busy identical across runs).  DVE floor (folds at 2
   elem/cycle/lane + 1x reduce) ~50us, DMA floor ~48us.
"""

import numpy as np

N, C, K, PARTS = 32, 256, 8192, 16
MAX_INIT = -100.0
NCORES = 8
NLOC = N // NCORES
P = 128
NCB = C // P

_CACHE = {}
_PATCHED = False

CONFIG = {
    "max_run": 3,  # split equal-cc runs longer than this many pairs
    "ft_bufs": 4, "m_bufs": 4, "acc_bufs": 4,
    "tp_bufs": 2, "mean_bufs": 2,
    # chunks in the first sub-DMA of slab 0 (rest in a second DMA): a
    # small first transfer starts the fold pipeline early; Tile subtile
    # deps let ops run as soon as their range lands
    "sub_chunks": 2,
    "reduce_split": 6,

}
NPAIR = PARTS // 2


def _patch_bass():
    """This container's walrus build accepts at most ONE sync-wait per
    instruction; split multi-wait instructions into single-wait NoOps."""
    global _PATCHED
    if _PATCHED:
        return
    import orjson
    import concourse.bass as bass

    orig = bass.Bass.to_json_bytes

    def patched(self):
        d = orjson.loads(orig(self))
        for fn in d.get("functions", []):
            for blk in fn.get("blocks", []):
                out, ctr = [], 0
                for ins in blk["instructions"]:
                    si = ins.get("sync_info") or {}
                    waits = si.get("on_wait") or []
                    if len(waits) > 1:
                        for w in waits[:-1]:
                            ctr += 1
                            out.append({
                                "debug": ins.get("debug"),
                                "engine": ins["engine"],
                                "ins": [],
                                "name": f"{ins['name']}-sw{ctr}",
                                "opcode": "NoOp",
                                "outs": [],
                                "sync_info": {"on_update": [],
                                              "on_wait": [w]},
                            })
                        si["on_wait"] = waits[-1:]
                    out.append(ins)
                blk["instructions"] = out
        return orjson.dumps(d)

    bass.Bass.to_json_bytes = patched
    _PATCHED = True


def _plan(chunk_counts):
    """Slab boundaries are aligned to runs of equal chunk count so each
    slab is ONE fold group: every fold op is a flat contiguous 2D slice
    covering all the slab's pairs (chunk = base + j*g + pair_idx,
    rank-major), minimizing DVE op count.  Long runs are split (SBUF /
    pipelining), so a slab may still hold several groups."""
    mr = CONFIG["max_run"]
    runs = []  # np_ per slab
    pos = 0
    while pos < NPAIR:
        cc = chunk_counts[pos]
        g = 1
        while pos + g < NPAIR and chunk_counts[pos + g] == cc:
            g += 1
        pos += g
        if not runs and g > 1:
            # tiny first slab so the fold pipeline starts after ~2
            # chunks of DMA instead of a whole multi-pair slab
            runs.append(1)
            g -= 1
        while g > mr:
            half = (g + 1) // 2
            h = min(half, mr)
            runs.append(h)
            g -= h
        runs.append(g)
    slabs = []  # (pair0, npairs, chunk0, nchunks, groups)
    chunk_order = []  # (pair, j) in device order
    p0 = 0
    c0 = 0
    for np_ in runs:
        ncsl = sum(chunk_counts[p0:p0 + np_])
        groups = []  # (pos, g, cc, slab-local chunk base)
        pos = 0
        cbase = 0
        while pos < np_:
            cc = chunk_counts[p0 + pos]
            g = 1
            while pos + g < np_ and chunk_counts[p0 + pos + g] == cc:
                g += 1
            groups.append((pos, g, cc, cbase))
            for j in range(cc):
                for i in range(g):
                    chunk_order.append((p0 + pos + i, j))
            cbase += g * cc
            pos += g
        slabs.append((p0, np_, c0, ncsl, groups))
        p0 += np_
        c0 += ncsl
    return slabs, chunk_order


def _build(nch, chunk_counts, ft_bufs):
    import concourse.bass as bass
    import concourse.tile as tile
    from concourse import mybir

    _patch_bass()
    bf = mybir.dt.bfloat16
    f32 = mybir.dt.float32

    nc = bass.Bass()
    feats_e = nc.declare_dram_parameter("feats", [NLOC, P, nch, 256], bf,
                                        isOutput=False)
    m_e = nc.declare_dram_parameter("m", [NLOC, P, nch, PARTS], bf,
                                    isOutput=False)
    ident_e = nc.declare_dram_parameter("ident", [P, P], bf, isOutput=False)
    out_e = nc.declare_dram_parameter("out", [NLOC, P, NCB, PARTS], f32,
                                      isOutput=True)

    slabs, _ = _plan(chunk_counts)

    with tile.TileContext(nc) as tc:
        with tc.tile_pool(name="ftp", bufs=ft_bufs) as ftp, \
             tc.tile_pool(name="mp", bufs=CONFIG["m_bufs"]) as mp, \
             tc.tile_pool(name="accp", bufs=CONFIG["acc_bufs"]) as accp, \
             tc.tile_pool(name="small", bufs=8) as smallp, \
             tc.tile_pool(name="tpp", bufs=CONFIG["tp_bufs"],
                          space="PSUM") as tpp, \
             tc.tile_pool(name="meanp", bufs=CONFIG["mean_bufs"],
                          space="PSUM") as meanp:
            ident = smallp.tile([P, P], bf, tag="ident")
            nc.scalar.dma_start(out=ident[:], in_=ident_e[:, :])
            mtalls = []
            for s in range(NLOC):
                mt = mp.tile([P, nch * PARTS], bf, tag="m",
                             name=f"m_{s}")
                nc.scalar.dma_start(out=mt[:], in_=m_e[s])
                mtalls.append(mt)

            def emit_reduce(tps, maxs, q0, q1):
                for cb in range(NCB):
                    nc.vector.tensor_reduce(
                        out=maxs[:, cb * PARTS + 2 * q0:
                                 cb * PARTS + 2 * q1],
                        in_=tps[cb][:, q0 * P:q1 * P].rearrange(
                            "p (t k) -> p t k", k=P // 2),
                        axis=mybir.AxisListType.X,
                        op=mybir.AluOpType.max,
                    )

            def emit_finish(s, pmean, tps, maxs, red_done):
                # late reduce + mean evac/transpose + combine + store;
                # deferred until after the NEXT sample's first slab so
                # the FIFO DVE queue never stalls on this sample's last
                # PE transposes at the sample boundary
                emit_reduce(tps, maxs, red_done, NPAIR)
                mean_sb = smallp.tile([PARTS, 256], bf, tag="mean_sb")
                nc.scalar.activation(
                    out=mean_sb[:], in_=pmean[:],
                    func=mybir.ActivationFunctionType.Copy,
                )
                res = smallp.tile([P, NCB * PARTS], f32, tag="res")
                mtp = meanp.tile([P, NCB * PARTS], bf, tag="mtp")
                for cb in range(NCB):
                    nc.tensor.transpose(
                        mtp[:, cb * PARTS:(cb + 1) * PARTS],
                        mean_sb[:, cb * P:(cb + 1) * P],
                        ident[0:PARTS, 0:PARTS],
                    )
                nc.vector.scalar_tensor_tensor(
                    out=res[:],
                    in0=maxs[:],
                    scalar=MAX_INIT,
                    in1=mtp[:],
                    op0=mybir.AluOpType.max,
                    op1=mybir.AluOpType.add,
                )
                nc.scalar.dma_start(out=out_e[s], in_=res[:])

            pending = None
            for s in range(NLOC):
                pmean = meanp.tile([PARTS, 256], f32, tag="pmean")
                ft_sl = []
                acc_sl = []
                m3 = mtalls[s][:].rearrange("p (t q) -> p t q", q=PARTS)
                tps = []
                for cb in range(NCB):
                    tpt = tpp.tile([P, NPAIR * P], bf, tag=f"tp{cb}",
                                   name=f"tp{cb}_{s}")
                    tps.append(tpt)
                maxs = smallp.tile([P, NCB * PARTS], f32, tag="maxs")
                rs = CONFIG["reduce_split"]
                red_done = 0
                for si, (p0, np_, c0, ncsl, groups) in enumerate(slabs):
                    if si == 1 and pending is not None:
                        pending()
                        pending = None
                    if ncsl == 0:
                        # slab of empty parts: nothing to load or fold;
                        # zero the acc slots so transposes read zeros
                        # (reference yields 0 there; host also re-zeros)
                        acc = accp.tile([P, np_ * 256], bf,
                                        tag=f"acc{si}",
                                        name=f"acc{si}_{s}")
                        acc_sl.append(acc)
                        nc.vector.memset(acc[:], 0.0)
                        for q in range(np_):
                            pt = p0 + q
                            for cb in range(NCB):
                                nc.tensor.transpose(
                                    tps[cb][:, pt * P:(pt + 1) * P],
                                    acc[:, q * 256 + cb * P:
                                        q * 256 + cb * P + P],
                                    ident[:],
                                )
                        continue
                    ft = ftp.tile([P, ncsl * 256], bf, tag=f"ft{si}")
                    nsub = CONFIG["sub_chunks"] if si == 0 else ncsl
                    nsub = min(nsub, ncsl)
                    nc.sync.dma_start(
                        out=ft[:, 0:nsub * 256],
                        in_=feats_e[s, :, c0:c0 + nsub, :])
                    if nsub < ncsl:
                        nc.sync.dma_start(
                            out=ft[:, nsub * 256:],
                            in_=feats_e[s, :, c0 + nsub:c0 + ncsl, :])
                    ft_sl.append(ft)

                    # MEAN: accumulate this slab's chunks into psum
                    for t in range(ncsl):
                        nc.tensor.matmul(
                            pmean[:],
                            lhsT=m3[:, c0 + t, :],
                            rhs=ft[:, t * 256:(t + 1) * 256],
                            start=(c0 + t == 0),
                            stop=(c0 + t == nch - 1),
                        )

                    # MAX: flat fold ops (chunk-rank-major group layout)
                    acc = accp.tile([P, np_ * 256], bf, tag=f"acc{si}")
                    acc_sl.append(acc)
                    for (pos, g, cc, cbase) in groups:
                        accv = acc[:, pos * 256:(pos + g) * 256]
                        if cc == 0:
                            nc.vector.memset(accv, 0.0)
                            continue

                        def ftg(j):
                            a = (cbase + j * g) * 256
                            return ft[:, a:a + g * 256]

                        if cc == 1:
                            nc.vector.tensor_copy(accv, ftg(0))
                        else:
                            for j in range(1, cc):
                                nc.vector.tensor_tensor(
                                    out=accv,
                                    in0=(ftg(0) if j == 1 else accv),
                                    in1=ftg(j),
                                    op=mybir.AluOpType.max,
                                )

                    # transpose this slab's parts now (acc slab ready)
                    for q in range(np_):
                        pt = p0 + q
                        for cb in range(NCB):
                            nc.tensor.transpose(
                                tps[cb][:, pt * P:(pt + 1) * P],
                                acc[:, q * 256 + cb * P:
                                    q * 256 + cb * P + P],
                                ident[:],
                            )
                    # partial reduce once the first >=rs pairs are
                    # transposed: DVE is FIFO, so emitting it here (not
                    # after all slabs) keeps only a small reduce on the
                    # tail after the last slab's folds
                    if red_done == 0 and p0 + np_ >= rs \
                            and p0 + np_ < NPAIR:
                        red_done = p0 + np_
                        emit_reduce(tps, maxs, 0, red_done)

                pending = (lambda s=s, pmean=pmean, tps=tps, maxs=maxs,
                           red_done=red_done:
                           emit_finish(s, pmean, tps, maxs, red_done))
            pending()
    return nc


def kernel(feats, part_labels, valid_mask, _timing=None):
    import ml_dtypes
    from concourse.bass_utils import run_bass_kernel_spmd

    feats = np.asarray(feats, dtype=np.float32)
    labels = np.asarray(part_labels).astype(np.int64)
    vm = np.asarray(valid_mask).astype(np.float32)
    bf16 = ml_dtypes.bfloat16

    order = np.argsort(labels, kind="stable")
    seg_len = np.bincount(labels, minlength=PARTS).astype(np.int64)
    # 64-row units; sorted parts pair up (2i, 2i+1) to minimize pair pad
    units = (-(-seg_len // (P // 2))).astype(np.int64)
    part_perm = np.argsort(units, kind="stable")  # slot j -> original part
    upair = units[part_perm]
    chunk_counts = [int(max(upair[2 * i], upair[2 * i + 1]))
                    for i in range(NPAIR)]  # chunks per PAIR
    nch = int(sum(chunk_counts))

    _, chunk_order = _plan(chunk_counts)
    cmax = max(chunk_counts) or 1
    nc_lut = np.zeros((NPAIR, cmax), dtype=np.int64)
    for idx, (pr, j) in enumerate(chunk_order):
        nc_lut[pr, j] = idx
    # slot j = pair j//2, half j%2: its rank r lives at
    # row nc_lut[pair, r//64]*128 + (j%2)*64 + r%64
    slot_of = np.empty(PARTS, dtype=np.int64)
    slot_of[part_perm] = np.arange(PARTS)
    seg_off = np.concatenate([[0], np.cumsum(seg_len)[:-1]]).astype(np.int64)
    ranks = np.arange(K, dtype=np.int64) - np.repeat(seg_off, seg_len)
    parts_of = np.repeat(np.arange(PARTS, dtype=np.int64), seg_len)
    slots = slot_of[parts_of]
    H = P // 2
    dest_row = (nc_lut[slots // 2, ranks // H] * P
                + (slots % 2) * H + ranks % H)

    KP = nch * P
    counts = vm @ (labels[:, None] == np.arange(PARTS)[None, :])
    inv_cnt = 1.0 / np.maximum(counts, 1.0)

    ft_rows = np.full((N, KP, C), MAX_INIT, dtype=np.float32)
    ft_rows[:, dest_row, :] = feats.transpose(0, 2, 1)[:, order, :]
    ft_dev = np.ascontiguousarray(
        ft_rows.reshape(N, nch, P, C).transpose(0, 2, 1, 3)
    ).astype(bf16)

    m_rows = np.zeros((N, KP, PARTS), dtype=np.float32)
    m_rows[:, dest_row, slots] = \
        vm[:, order] * inv_cnt[:, parts_of]
    m_dev = np.ascontiguousarray(
        m_rows.reshape(N, nch, P, PARTS).transpose(0, 2, 1, 3)
    ).astype(bf16)

    ident = np.eye(P, dtype=bf16)

    # ft pool is nch*0.5KB/partition per buf set; stay under ~155KB total
    ft_bufs = min(CONFIG["ft_bufs"], max(2, 140 // max(1, nch // 2)))
    key = (nch, tuple(chunk_counts), ft_bufs,
           tuple(sorted(CONFIG.items())))
    if key not in _CACHE:
        _CACHE[key] = _build(nch, chunk_counts, ft_bufs)
    nc = _CACHE[key]

    in_maps = [
        {
            "feats": ft_dev[i * NLOC:(i + 1) * NLOC],
            "m": m_dev[i * NLOC:(i + 1) * NLOC],
            "ident": ident,
        }
        for i in range(NCORES)
    ]
    res = run_bass_kernel_spmd(
        nc, in_maps, core_ids=list(range(NCORES)),
        **({} if _timing is None else _timing),
    )
    if _timing is not None:
        kernel.last_result = res

    out = np.concatenate([r["out"] for r in res.results], axis=0)
    out = out.transpose(0, 2, 1, 3).reshape(N, C, PARTS)
    full = np.empty_like(out)
    full[:, :, part_perm] = out
    empty = np.where(seg_len == 0)[0]
    if empty.size:
        full[:, :, empty] = 0.0
    return full



# revision 37
# speedup vs baseline: 1.0728x; 1.0728x over previous
"""AnchorPatchPooling Trainium2 kernel.

Math (per sample n, channel c, part p):
  out[n,c,p] = sum_{k: lab[k]=p} feats[n,c,k]*vm[n,k] / max(count[n,p],1)
             + where(patch_count[p]>0, max(-100, max_{k: lab[k]=p} feats[n,c,k]), 0)

Design (PE-matmul mean + transposed-layout max):
 - Data-parallel over n across 8 cores (4 samples/core), no collectives.
 - Host re-lays feats per sample as fT[k, c]: k sorted part-major (parts
   ordered by chunk count), each part's segment padded with -100 rows to
   a multiple of 128, swizzled to [128, nch, 256] chunks, bf16.
 - MEAN on the TensorEngine: host builds M[k, p] = vm[n,k]/max(cnt,1)
   one-hot bf16 (pad rows 0).  Per 128-row chunk k: matmul(psum[16,256],
   lhsT=M_chunk[128,16], rhs=fT_chunk[128,256], accumulate) - the
   partition-dim contraction IS the segment scatter-add, and the mask
   multiply + count division ride along in M for free.  ACT evacuates
   psum, 2 small PE transposes give [c,p].
 - MAX on DVE+PE: per part, sequential wide bf16 tensor_tensor folds
   (2x_1P mode) collapse its chunks onto one [128,256] acc slot; PE
   transposes each [128k,128c] block to bf16 PSUM; 1x tensor_reduce
   per (sample, cb) over [p,t,64] finishes across the k-remnant, split
   in two (early pairs mid-stream, rest on the tail) since the DVE
   queue is FIFO.
 - One fused scalar_tensor_tensor does (max clamp -100) + meanT.
 - Pipelining (the DVE queue paces the kernel; DMA ~48us busy/engine,
   DVE ~56-62us busy):
   * slabs are runs of equal chunk-count (split >max_run) so each fold
     op is a flat contiguous 2D slice across all the slab's pairs;
     a 1-pair first slab + a 2-chunk first sub-DMA start folds ~9.5us
     in (Tile subtile deps release ops as their byte-range lands);
   * ft slabs ride the sync HWDGE ring; ident + all 4 M tiles (issued
     up-front) + out DMAs ride the scalar ring, so the per-sample mean
     evac COPY on ACT can't head-of-line-block later descriptor gens;
   * balanced slab sizes ([1,2,3,2] pairs) avoid a 2.9MB monster slab
     whose delivery Tile reorders behind the next sample's slabs.
 - Measured: ~77-81us (exec time varies with a bimodal device clock
   state; engine-busy identical across runs).  DVE floor (folds at 2
   elem/cycle/lane + 1x reduce) ~50us, DMA floor ~48us.
"""

import numpy as np

N, C, K, PARTS = 32, 256, 8192, 16
MAX_INIT = -100.0
NCORES = 8
NLOC = N // NCORES
P = 128
NCB = C // P

_CACHE = {}
_PATCHED = False

CONFIG = {
    "max_run": 3,  # split equal-cc runs longer than this many pairs
    "ft_bufs": 4, "m_bufs": 4, "acc_bufs": 4,
    "tp_bufs": 2, "mean_bufs": 2,
    # chunks in the first sub-DMA of slab 0 (rest in a second DMA): a
    # small first transfer starts the fold pipeline early; Tile subtile
    # deps let ops run as soon as their range lands
    "sub_chunks": 2,
    "reduce_split": 3,

}
NPAIR = PARTS // 2


def _patch_bass():
    """This container's walrus build accepts at most ONE sync-wait per
    instruction; split multi-wait instructions into single-wait NoOps."""
    global _PATCHED
    if _PATCHED:
        return
    import orjson
    import concourse.bass as bass

    orig = bass.Bass.to_json_bytes

    def patched(self):
        d = orjson.loads(orig(self))
        for fn in d.get("functions", []):
            for blk in fn.get("blocks", []):
                out, ctr = [], 0
                for ins in blk["instructions"]:
                    si = ins.get("sync_info") or {}
                    waits = si.get("on_wait") or []
                    if len(waits) > 1:
                        for w in waits[:-1]:
                            ctr += 1
                            out.append({
                                "debug": ins.get("debug"),
                                "engine": ins["engine"],
                                "ins": [],
                                "name": f"{ins['name']}-sw{ctr}",
                                "opcode": "NoOp",
                                "outs": [],
                                "sync_info": {"on_update": [],
                                              "on_wait": [w]},
                            })
                        si["on_wait"] = waits[-1:]
                    out.append(ins)
                blk["instructions"] = out
        return orjson.dumps(d)

    bass.Bass.to_json_bytes = patched
    _PATCHED = True


def _plan(chunk_counts):
    """Slab boundaries are aligned to runs of equal chunk count so each
    slab is ONE fold group: every fold op is a flat contiguous 2D slice
    covering all the slab's pairs (chunk = base + j*g + pair_idx,
    rank-major), minimizing DVE op count.  Long runs are split (SBUF /
    pipelining), so a slab may still hold several groups."""
    mr = CONFIG["max_run"]
    runs = []  # np_ per slab
    pos = 0
    while pos < NPAIR:
        cc = chunk_counts[pos]
        g = 1
        while pos + g < NPAIR and chunk_counts[pos + g] == cc:
            g += 1
        pos += g
        if not runs and g > 1:
            # tiny first slab so the fold pipeline starts after ~2
            # chunks of DMA instead of a whole multi-pair slab
            runs.append(1)
            g -= 1
        while g > mr:
            half = (g + 1) // 2
            h = min(half, mr)
            runs.append(h)
            g -= h
        runs.append(g)
    slabs = []  # (pair0, npairs, chunk0, nchunks, groups)
    chunk_order = []  # (pair, j) in device order
    p0 = 0
    c0 = 0
    for np_ in runs:
        ncsl = sum(chunk_counts[p0:p0 + np_])
        groups = []  # (pos, g, cc, slab-local chunk base)
        pos = 0
        cbase = 0
        while pos < np_:
            cc = chunk_counts[p0 + pos]
            g = 1
            while pos + g < np_ and chunk_counts[p0 + pos + g] == cc:
                g += 1
            groups.append((pos, g, cc, cbase))
            for j in range(cc):
                for i in range(g):
                    chunk_order.append((p0 + pos + i, j))
            cbase += g * cc
            pos += g
        slabs.append((p0, np_, c0, ncsl, groups))
        p0 += np_
        c0 += ncsl
    return slabs, chunk_order


def _build(nch, chunk_counts, ft_bufs):
    import concourse.bass as bass
    import concourse.tile as tile
    from concourse import mybir

    _patch_bass()
    bf = mybir.dt.bfloat16
    f32 = mybir.dt.float32

    nc = bass.Bass()
    feats_e = nc.declare_dram_parameter("feats", [NLOC, P, nch, 256], bf,
                                        isOutput=False)
    m_e = nc.declare_dram_parameter("m", [NLOC, P, nch, PARTS], bf,
                                    isOutput=False)
    ident_e = nc.declare_dram_parameter("ident", [P, P], bf, isOutput=False)
    out_e = nc.declare_dram_parameter("out", [NLOC, P, NCB, PARTS], f32,
                                      isOutput=True)

    slabs, _ = _plan(chunk_counts)

    with tile.TileContext(nc) as tc:
        with tc.tile_pool(name="ftp", bufs=ft_bufs) as ftp, \
             tc.tile_pool(name="mp", bufs=CONFIG["m_bufs"]) as mp, \
             tc.tile_pool(name="accp", bufs=CONFIG["acc_bufs"]) as accp, \
             tc.tile_pool(name="small", bufs=8) as smallp, \
             tc.tile_pool(name="tpp", bufs=CONFIG["tp_bufs"],
                          space="PSUM") as tpp, \
             tc.tile_pool(name="meanp", bufs=CONFIG["mean_bufs"],
                          space="PSUM") as meanp:
            ident = smallp.tile([P, P], bf, tag="ident")
            nc.scalar.dma_start(out=ident[:], in_=ident_e[:, :])
            mtalls = []
            for s in range(NLOC):
                mt = mp.tile([P, nch * PARTS], bf, tag="m",
                             name=f"m_{s}")
                nc.scalar.dma_start(out=mt[:], in_=m_e[s])
                mtalls.append(mt)

            def emit_reduce(tps, maxs, q0, q1):
                for cb in range(NCB):
                    nc.vector.tensor_reduce(
                        out=maxs[:, cb * PARTS + 2 * q0:
                                 cb * PARTS + 2 * q1],
                        in_=tps[cb][:, q0 * P:q1 * P].rearrange(
                            "p (t k) -> p t k", k=P // 2),
                        axis=mybir.AxisListType.X,
                        op=mybir.AluOpType.max,
                    )

            def emit_finish(s, pmean, tps, maxs, red_done):
                # late reduce + mean evac/transpose + combine + store;
                # deferred until after the NEXT sample's first slab so
                # the FIFO DVE queue never stalls on this sample's last
                # PE transposes at the sample boundary
                emit_reduce(tps, maxs, red_done, NPAIR)
                mean_sb = smallp.tile([PARTS, 256], bf, tag="mean_sb")
                nc.scalar.activation(
                    out=mean_sb[:], in_=pmean[:],
                    func=mybir.ActivationFunctionType.Copy,
                )
                res = smallp.tile([P, NCB * PARTS], f32, tag="res")
                mtp = meanp.tile([P, NCB * PARTS], bf, tag="mtp")
                for cb in range(NCB):
                    nc.tensor.transpose(
                        mtp[:, cb * PARTS:(cb + 1) * PARTS],
                        mean_sb[:, cb * P:(cb + 1) * P],
                        ident[0:PARTS, 0:PARTS],
                    )
                nc.vector.scalar_tensor_tensor(
                    out=res[:],
                    in0=maxs[:],
                    scalar=MAX_INIT,
                    in1=mtp[:],
                    op0=mybir.AluOpType.max,
                    op1=mybir.AluOpType.add,
                )
                nc.scalar.dma_start(out=out_e[s], in_=res[:])

            pending = None
            for s in range(NLOC):
                pmean = meanp.tile([PARTS, 256], f32, tag="pmean")
                ft_sl = []
                acc_sl = []
                m3 = mtalls[s][:].rearrange("p (t q) -> p t q", q=PARTS)
                tps = []
                for cb in range(NCB):
                    tpt = tpp.tile([P, NPAIR * P], bf, tag=f"tp{cb}",
                                   name=f"tp{cb}_{s}")
                    tps.append(tpt)
                maxs = smallp.tile([P, NCB * PARTS], f32, tag="maxs")
                rs = CONFIG["reduce_split"]
                red_done = 0
                for si, (p0, np_, c0, ncsl, groups) in enumerate(slabs):
                    if si == 1 and pending is not None:
                        pending()
                        pending = None
                    if ncsl == 0:
                        # slab of empty parts: nothing to load or fold;
                        # zero the acc slots so transposes read zeros
                        # (reference yields 0 there; host also re-zeros)
                        acc = accp.tile([P, np_ * 256], bf,
                                        tag=f"acc{si}",
                                        name=f"acc{si}_{s}")
                        acc_sl.append(acc)
                        nc.vector.memset(acc[:], 0.0)
                        for q in range(np_):
                            pt = p0 + q
                            for cb in range(NCB):
                                nc.tensor.transpose(
                                    tps[cb][:, pt * P:(pt + 1) * P],
                                    acc[:, q * 256 + cb * P:
                                        q * 256 + cb * P + P],
                                    ident[:],
                                )
                        continue
                    ft = ftp.tile([P, ncsl * 256], bf, tag=f"ft{si}")
                    nsub = CONFIG["sub_chunks"] if si == 0 else ncsl
                    nsub = min(nsub, ncsl)
                    nc.sync.dma_start(
                        out=ft[:, 0:nsub * 256],
                        in_=feats_e[s, :, c0:c0 + nsub, :])
                    if nsub < ncsl:
                        nc.sync.dma_start(
                            out=ft[:, nsub * 256:],
                            in_=feats_e[s, :, c0 + nsub:c0 + ncsl, :])
                    ft_sl.append(ft)

                    # MEAN: accumulate this slab's chunks into psum
                    for t in range(ncsl):
                        nc.tensor.matmul(
                            pmean[:],
                            lhsT=m3[:, c0 + t, :],
                            rhs=ft[:, t * 256:(t + 1) * 256],
                            start=(c0 + t == 0),
                            stop=(c0 + t == nch - 1),
                        )

                    # MAX: flat fold ops (chunk-rank-major group layout)
                    acc = accp.tile([P, np_ * 256], bf, tag=f"acc{si}")
                    acc_sl.append(acc)
                    for (pos, g, cc, cbase) in groups:
                        accv = acc[:, pos * 256:(pos + g) * 256]
                        if cc == 0:
                            nc.vector.memset(accv, 0.0)
                            continue

                        def ftg(j):
                            a = (cbase + j * g) * 256
                            return ft[:, a:a + g * 256]

                        if cc == 1:
                            nc.vector.tensor_copy(accv, ftg(0))
                        else:
                            for j in range(1, cc):
                                nc.vector.tensor_tensor(
                                    out=accv,
                                    in0=(ftg(0) if j == 1 else accv),
                                    in1=ftg(j),
                                    op=mybir.AluOpType.max,
                                )

                    # transpose this slab's parts now (acc slab ready)
                    for q in range(np_):
                        pt = p0 + q
                        for cb in range(NCB):
                            nc.tensor.transpose(
                                tps[cb][:, pt * P:(pt + 1) * P],
                                acc[:, q * 256 + cb * P:
                                    q * 256 + cb * P + P],
                                ident[:],
                            )
                    # partial reduce once the first >=rs pairs are
                    # transposed: DVE is FIFO, so emitting it here (not
                    # after all slabs) keeps only a small reduce on the
                    # tail after the last slab's folds
                    if red_done == 0 and p0 + np_ >= rs \
                            and p0 + np_ < NPAIR:
                        red_done = p0 + np_
                        emit_reduce(tps, maxs, 0, red_done)

                pending = (lambda s=s, pmean=pmean, tps=tps, maxs=maxs,
                           red_done=red_done:
                           emit_finish(s, pmean, tps, maxs, red_done))
            pending()
    return nc


def kernel(feats, part_labels, valid_mask, _timing=None):
    import ml_dtypes
    from concourse.bass_utils import run_bass_kernel_spmd

    feats = np.asarray(feats, dtype=np.float32)
    labels = np.asarray(part_labels).astype(np.int64)
    vm = np.asarray(valid_mask).astype(np.float32)
    bf16 = ml_dtypes.bfloat16

    order = np.argsort(labels, kind="stable")
    seg_len = np.bincount(labels, minlength=PARTS).astype(np.int64)
    # 64-row units; sorted parts pair up (2i, 2i+1) to minimize pair pad
    units = (-(-seg_len // (P // 2))).astype(np.int64)
    part_perm = np.argsort(units, kind="stable")  # slot j -> original part
    upair = units[part_perm]
    chunk_counts = [int(max(upair[2 * i], upair[2 * i + 1]))
                    for i in range(NPAIR)]  # chunks per PAIR
    nch = int(sum(chunk_counts))

    _, chunk_order = _plan(chunk_counts)
    cmax = max(chunk_counts) or 1
    nc_lut = np.zeros((NPAIR, cmax), dtype=np.int64)
    for idx, (pr, j) in enumerate(chunk_order):
        nc_lut[pr, j] = idx
    # slot j = pair j//2, half j%2: its rank r lives at
    # row nc_lut[pair, r//64]*128 + (j%2)*64 + r%64
    slot_of = np.empty(PARTS, dtype=np.int64)
    slot_of[part_perm] = np.arange(PARTS)
    seg_off = np.concatenate([[0], np.cumsum(seg_len)[:-1]]).astype(np.int64)
    ranks = np.arange(K, dtype=np.int64) - np.repeat(seg_off, seg_len)
    parts_of = np.repeat(np.arange(PARTS, dtype=np.int64), seg_len)
    slots = slot_of[parts_of]
    H = P // 2
    dest_row = (nc_lut[slots // 2, ranks // H] * P
                + (slots % 2) * H + ranks % H)

    KP = nch * P
    counts = vm @ (labels[:, None] == np.arange(PARTS)[None, :])
    inv_cnt = 1.0 / np.maximum(counts, 1.0)

    ft_rows = np.full((N, KP, C), MAX_INIT, dtype=np.float32)
    ft_rows[:, dest_row, :] = feats.transpose(0, 2, 1)[:, order, :]
    ft_dev = np.ascontiguousarray(
        ft_rows.reshape(N, nch, P, C).transpose(0, 2, 1, 3)
    ).astype(bf16)

    m_rows = np.zeros((N, KP, PARTS), dtype=np.float32)
    m_rows[:, dest_row, slots] = \
        vm[:, order] * inv_cnt[:, parts_of]
    m_dev = np.ascontiguousarray(
        m_rows.reshape(N, nch, P, PARTS).transpose(0, 2, 1, 3)
    ).astype(bf16)

    ident = np.eye(P, dtype=bf16)

    # ft pool is nch*0.5KB/partition per buf set; stay under ~155KB total
    ft_bufs = min(CONFIG["ft_bufs"], max(2, 140 // max(1, nch // 2)))
    key = (nch, tuple(chunk_counts), ft_bufs,
           tuple(sorted(CONFIG.items())))
    if key not in _CACHE:
        _CACHE[key] = _build(nch, chunk_counts, ft_bufs)
    nc = _CACHE[key]

    in_maps = [
        {
            "feats": ft_dev[i * NLOC:(i + 1) * NLOC],
            "m": m_dev[i * NLOC:(i + 1) * NLOC],
            "ident": ident,
        }
        for i in range(NCORES)
    ]
    res = run_bass_kernel_spmd(
        nc, in_maps, core_ids=list(range(NCORES)),
        **({} if _timing is None else _timing),
    )
    if _timing is not None:
        kernel.last_result = res

    out = np.concatenate([r["out"] for r in res.results], axis=0)
    out = out.transpose(0, 2, 1, 3).reshape(N, C, PARTS)
    full = np.empty_like(out)
    full[:, :, part_perm] = out
    empty = np.where(seg_len == 0)[0]
    if empty.size:
        full[:, :, empty] = 0.0
    return full

